# revision 39
# baseline (speedup 1.0000x reference)
"""CBOW forward kernel for one TRN2 chip (8 NeuronCores), tensor-parallel on vocab.

Math (matches the reference):
    embed[b, c, :] = emb_W.T[contexts[b, c]] + emb_b          # gather
    out = embed.reshape(B, CTX*EMB) @ fc_W.T + fc_b           # [B, VOCAB]

No collectives: the vocab dim is sharded 8 ways (fc_W rows / fc_b /
output rows); the emb table is replicated and EVERY core gathers the FULL
batch locally, so there is no AllGather (the CC engine's ~47us mesh-init
made a collective-based prologue idle until ~90us).

Per-core device schedule:
  1. fc_W (fp8, 6.4 MiB) DMAs into SBUF in 7 chunks and stays resident.
     Chunk 0 loads up front; 1-6 are staged mid-kernel (from the gpsimd
     gather stream and after the first output DMAs) because early bulk
     loads clog the DMA queues and delay gather-data arrival by ~5-8us.
  2. The batch (2048 cols) is processed in 6 chunks of 2/2/3/3/3/3
     128-col blocks.  Per chunk: one indirect-DMA gather per (block, ctx)
     -- the hw consumes ONE index per partition per call, ~1.4us each on
     gpsimd, which paces the whole kernel -- pulls bf16 emb rows
     (pre-scaled by 512 on host); PE transposes (bf16, 1 cyc/row) + DVE
     casts produce the K-major fp8 rhs; then 98 vocab tiles x 2 fp8
     DoubleRow matmuls (K=512 as 2x256) accumulate in PSUM.  Chunk c+1's
     gathers overlap chunk c's matmuls; short chunks first (early matmul
     start) and short chunks last (less matmul work left when the gather
     stream ends at ~191us) minimize the makespan.  A burst of garbage
     warmup matmuls keeps the PE clock ramping before chunk 0 lands.
  3. PSUM drain (alternating scalar/vector) fuses the 2^-18 descale and
     bias add into bf16; output DMAs are batched 7 vocab tiles at a time
     (sync queue) into a [98, 128, 2048] DRAM layout that reshapes to
     [VSHARD, BATCH] on the host with no transpose; the final groups are
     split so the last DMAs start draining before the last drains finish.

Both matmul operands are pre-scaled by 512 (2^9) so fp8e4 stays out of
its denormal range; emb_b and fc_b are folded on the host into one
effective f32 bias fc_be = fc_W @ tile(emb_b, CTX) + fc_b.
"""

import os

import numpy as np

import concourse.bacc as bacc
import concourse.bass as bass
import concourse.mybir as mybir
import concourse.tile as tile
from concourse.bass_utils import run_bass_kernel_spmd

# Problem shape (hardcoded per harness contract).
VOCAB = 100000
CTX = 8
EMB = 64
BATCH = 2048
K = CTX * EMB            # 512 contraction dim
NCORES = 8
VSHARD = 12544           # 98 * 128, vocab rows per core (padded)
VPAD = VSHARD * NCORES   # 100352
NVT = VSHARD // 128      # 98 vocab tiles per core
NCHUNK = 7               # fc_W DMA chunks
VT_PER_CHUNK = NVT // NCHUNK   # 14 vocab tiles per chunk
CHUNK_COLS = VT_PER_CHUNK * 128  # 1792

# batch chunks: 128-col blocks each.  The gather is 1 indirect call per
# (block, ctx) at ~1.4us on gpsimd (the hw reads ONE index per partition
# per call), so the stream paces the pipeline; two short chunks up front
# start the matmuls early, 512-col chunks later maximize N-efficiency.
BCHUNKS = [
    (0, 1),
    (2, 3),
    (4, 5, 6),
    (7, 8, 9),
    (10, 11, 12),
    (13, 14, 15),
]
NWARM = 24               # idle-PE warmup matmuls while chunk 0 gathers land

SCALE = 512.0            # 2^9 per fp8 operand (avoid e4m3 denormals)
DESCALE = 2.0 ** -18     # undone in the PSUM drain

F32 = mybir.dt.float32
BF16 = mybir.dt.bfloat16
I32 = mybir.dt.int32
FP8 = mybir.dt.float8e4
NP_FP8 = mybir.dt.np(FP8)
NP_BF16 = mybir.dt.np(BF16)
OUT_DT = BF16            # output quantization: rel err ~1.4e-2 < 2e-2 gate

_CACHE = {}


def _install_trace_hook():
    """Provide the missing antenv.axon_hooks module so trace=True works.

    The agent image's antenv lacks axon_hooks; recreate it and install the
    ctypes NTFF hook from trn_boot.  Degrades silently on any failure.
    """
    import sys
    import types

    try:
        if "antenv.axon_hooks" not in sys.modules:
            mod = types.ModuleType("antenv.axon_hooks")
            mod._hook = None
            mod.set_axon_ntff_profile_hook = lambda h: setattr(mod, "_hook", h)
            mod.get_axon_ntff_profile_hook = lambda: mod._hook
            sys.modules["antenv.axon_hooks"] = mod
            import antenv

            antenv.axon_hooks = mod
        mod = sys.modules["antenv.axon_hooks"]
        if mod.get_axon_ntff_profile_hook() is None:
            if "/root/.axon_site/trn_agent_boot" not in sys.path:
                sys.path.insert(0, "/root/.axon_site/trn_agent_boot")
            import trn_boot

            mod.set_axon_ntff_profile_hook(
                trn_boot._ntff_profile_via_ctypes("/opt/axon/libaxon_pjrt.so")
            )
        return True
    except Exception as e:  # pragma: no cover
        print(f"trace hook install failed: {type(e).__name__}: {e}")
        return False


def _build_nc():
    nc = bacc.Bacc(
        "TRN2", target_bir_lowering=False, debug=False, num_devices=NCORES
    )
    # idx_my[p, j] = contexts[(j//8)*128 + p, j%8]; col block (j//8), ctx j%8
    idx_my = nc.declare_dram_parameter("idx_my", [128, 128], I32, isOutput=False)
    emb_wt = nc.declare_dram_parameter("emb_wt", [VOCAB, EMB], BF16, isOutput=False)
    ident_d = nc.declare_dram_parameter("ident_d", [128, 128], BF16, isOutput=False)
    fc_w = nc.declare_dram_parameter(
        "fc_w", [NCHUNK, 128, 4, CHUNK_COLS], FP8, isOutput=False
    )
    fc_be = nc.declare_dram_parameter("fc_be", [128, NVT], F32, isOutput=False)
    # out[v, p, d] = logits_scaled[vocab row v*128+p, batch col d]
    out = nc.declare_dram_parameter("out", [NVT, 128, BATCH], OUT_DT, isOutput=True)

    DR = mybir.MatmulPerfMode.DoubleRow

    with tile.TileContext(nc) as tc:
        with tc.tile_pool(name="const", bufs=1) as const:
            idx_sb = const.tile([128, 128], I32, tag="idx", name="idx_sb")
            nc.sync.dma_start(out=idx_sb[:], in_=idx_my[:])
            ident = const.tile([128, 128], BF16, tag="ident", name="ident")
            nc.sync.dma_start(out=ident[:], in_=ident_d[:])
            fcbe_sb = const.tile([128, NVT], F32, tag="fcbe", name="fcbe_sb")
            nc.sync.dma_start(out=fcbe_sb[:], in_=fc_be[:])
            # fc_W fp8 shard: fully SBUF-resident (7 x 917 KiB chunks).
            # fcw[ci][p, ksub, col] = fc_W.T_scaled[ksub*128+p, ci*1792+col]
            # Chunks 0-2 load up front; 3-6 when the first matmul chunk
            # starts, so the gather descriptors aren't stuck behind 6.4 MiB.
            fcw = []
            for ci in range(NCHUNK):
                t = const.tile(
                    [128, 4, CHUNK_COLS], FP8, tag=f"fcw{ci}", name=f"fcw{ci}"
                )
                if ci < 1:
                    nc.scalar.dma_start(out=t[:], in_=fc_w[ci])
                fcw.append(t)

            # warm the ACT Identity table before the main loop needs it
            actwarm = const.tile([128, 1], F32, tag="actwarm", name="actwarm")
            nc.scalar.activation(
                out=actwarm[:],
                in_=fcbe_sb[:, 0:1],
                func=mybir.ActivationFunctionType.Identity,
                bias=fcbe_sb[:, 0:1],
            )

            with (
                tc.tile_pool(name="rawp", bufs=4) as rawp,
                tc.tile_pool(name="embp", bufs=4) as embp,
                tc.tile_pool(name="tpsum", bufs=2, space="PSUM") as tpp,
                tc.tile_pool(name="mpsum", bufs=6, space="PSUM") as mps,
                tc.tile_pool(name="outp", bufs=4) as osbp,
            ):
                # PE p-state warmup: one PSUM accumulation group of garbage
                # bf16 matmuls keeps the tensor clock ramping while chunk 0's
                # gather data lands.  Never read back.
                wps = mps.tile([128, 512], F32, tag="mps", name="mps")
                for i in range(NWARM):
                    nc.tensor.matmul(
                        out=wps[:, 0:128],
                        lhsT=ident[:],
                        rhs=ident[:],
                        start=(i == 0),
                        stop=(i == NWARM - 1),
                    )
                col0 = 0
                for c, mms in enumerate(BCHUNKS):
                    nmm = len(mms)
                    ncols = 128 * nmm
                    # gather: raw[p, (gi*8+cc)*64 + e] = emb_scaled[contexts[
                    #   (mms[gi])*128 + p, cc], e].  One row per partition
                    #   per call: the hw reads a single index per partition
                    #   and copies a contiguous run, so multi-index offset
                    #   APs silently gather rows base..base+k-1 instead.
                    raw = rawp.tile([128, 32 * EMB], BF16, tag="raw", name="raw")
                    for gi, mm in enumerate(mms):
                        for cc in range(8):
                            j = mm * 8 + cc
                            nc.gpsimd.indirect_dma_start(
                                out=raw[:, (gi * 8 + cc) * EMB : (gi * 8 + cc + 1) * EMB],
                                out_offset=None,
                                in_=emb_wt[:],
                                in_offset=bass.IndirectOffsetOnAxis(
                                    ap=idx_sb[:, j : j + 1], axis=0
                                ),
                            )
                            if c == 0 and gi * 8 + cc in (11, 15):
                                # fcw 1-2 issue mid-gather-stream from this
                                # queue: early enough for the matmul deadline,
                                # late enough not to clog the DMA queues that
                                # chunk 0/1's gather data drains through.
                                cl = 1 if gi * 8 + cc == 11 else 2
                                nc.gpsimd.dma_start(out=fcw[cl][:], in_=fc_w[cl])
                    # transpose to K-major fp8:
                    # embT[j][q, i, gi*128+p] = emb_scaled.T[(2j+i)*128+q,
                    #   batch col mms[gi]*128+p]
                    embT = [
                        embp.tile(
                            [128, 2, 512], FP8, tag=f"eT{j}", name=f"embT{j}"
                        )
                        for j in range(2)
                    ]
                    for gi in range(nmm):
                        for t in range(4):
                            psT = tpp.tile(
                                [128, 1024], BF16, tag="psT", name="psT"
                            )
                            nc.tensor.transpose(
                                psT[:, 0:128],
                                raw[
                                    :,
                                    (gi * 8 + 2 * t) * EMB
                                    : (gi * 8 + 2 * t + 2) * EMB,
                                ],
                                ident[:],
                            )
                            nc.vector.tensor_copy(
                                out=embT[t // 2][
                                    :, t % 2, gi * 128 : (gi + 1) * 128
                                ],
                                in_=psT[:, 0:128],
                            )


                    # matmul sweep: 98 vocab tiles x 2 fp8 DR matmuls
                    last_chunk = c == len(BCHUNKS) - 1
                    for vg in range(NVT // 7):
                        osb = osbp.tile(
                            [128, 7, 512], OUT_DT, tag="osb", name="osb"
                        )
                        split = last_chunk and vg >= 12
                        for vv in range(7):
                            v = vg * 7 + vv
                            ci, vt = divmod(v, VT_PER_CHUNK)
                            ps = mps.tile([128, 512], F32, tag="mps", name="mps")
                            for j in range(2):
                                nc.tensor.matmul(
                                    out=ps[:, 0:ncols],
                                    lhsT=fcw[ci][
                                        :, 2 * j : 2 * j + 2,
                                        vt * 128 : (vt + 1) * 128,
                                    ],
                                    rhs=embT[j][:, :, 0:ncols],
                                    start=(j == 0),
                                    stop=(j == 1),
                                    perf_mode=DR,
                                )
                            if (v + c) % 2 == 0:
                                nc.scalar.activation(
                                    out=osb[:, vv, 0:ncols],
                                    in_=ps[:, 0:ncols],
                                    func=mybir.ActivationFunctionType.Identity,
                                    bias=fcbe_sb[:, v : v + 1],
                                    scale=DESCALE,
                                )

                            else:
                                nc.vector.tensor_scalar(
                                    out=osb[:, vv, 0:ncols],
                                    in0=ps[:, 0:ncols],
                                    scalar1=DESCALE,
                                    scalar2=fcbe_sb[:, v : v + 1],
                                    op0=mybir.AluOpType.mult,
                                    op1=mybir.AluOpType.add,
                                )
                            if split and vv == 3:
                                # kernel-tail trim: fire the first half of the
                                # final output groups as soon as 4 drains land
                                nc.sync.dma_start(
                                    out=out[
                                        vg * 7 : vg * 7 + 4, :,
                                        col0 : col0 + ncols,
                                    ].rearrange("v p d -> p v d"),
                                    in_=osb[:, 0:4, 0:ncols],
                                )
                        nc.sync.dma_start(
                            out=out[
                                vg * 7 + (4 if split else 0) : vg * 7 + 7,
                                :, col0 : col0 + ncols,
                            ].rearrange("v p d -> p v d"),
                            in_=osb[:, (4 if split else 0) : 7, 0:ncols],
                        )
                        if c == 0 and vg < 4:
                            # fcw 3-6 stream after chunk 0's first output
                            # DMAs -- paced issue keeps the queues clear for
                            # the ongoing gather stream.
                            nc.sync.dma_start(
                                out=fcw[3 + vg][:], in_=fc_w[3 + vg]
                            )
                    col0 += ncols
    nc.compile()
    return nc


def _prep_inputs(contexts, emb_W, emb_b, fc_W, fc_b):
    contexts = np.asarray(contexts)
    emb_W = np.asarray(emb_W, dtype=np.float32)
    emb_b = np.asarray(emb_b, dtype=np.float32)
    fc_W = np.asarray(fc_W, dtype=np.float32)
    fc_b = np.asarray(fc_b, dtype=np.float32)

    # idx2d[j, p] = contexts[(j//8)*128 + p, j%8]; device gathers column
    # block j//8 (128 batch rows), ctx position j%8, natural batch order
    idx2d = (
        contexts.astype(np.int64).reshape(BATCH // 128, 128, CTX)
        .transpose(0, 2, 1).reshape(BATCH // 128 * CTX, 128)
    )
    idx_all = np.ascontiguousarray(idx2d.T.astype(np.int32))  # [128, 128]
    emb_wt = np.ascontiguousarray((emb_W.T * SCALE).astype(NP_BF16))
    ident = np.eye(128, dtype=np.float32).astype(NP_BF16)

    # effective bias: fc_be = fc_W @ tile(emb_b, CTX) + fc_b  (padded, true scale)
    emb_b_t = np.tile(emb_b, CTX)
    fc_be_full = (
        fc_W.astype(np.float64) @ emb_b_t.astype(np.float64)
        + fc_b.astype(np.float64)
    ).astype(np.float32)
    fc_be_pad = np.zeros(VPAD, dtype=np.float32)
    fc_be_pad[:VOCAB] = fc_be_full

    # fc_W.T padded to VPAD cols, scaled into fp8 range, chunked per-core
    fcT = np.zeros((K, VPAD), dtype=np.float32)
    fcT[:, :VOCAB] = fc_W.T
    fcT = np.clip(fcT * SCALE, -240.0, 240.0).astype(NP_FP8)

    in_maps = []
    for s in range(NCORES):
        shard = fcT[:, s * VSHARD : (s + 1) * VSHARD]
        fc_host = np.ascontiguousarray(
            shard.reshape(4, 128, NCHUNK, CHUNK_COLS).transpose(2, 1, 0, 3)
        )
        be = np.ascontiguousarray(
            fc_be_pad[s * VSHARD : (s + 1) * VSHARD].reshape(NVT, 128).T
        )
        in_maps.append(
            {
                "idx_my": idx_all,
                "emb_wt": emb_wt,
                "ident_d": ident,
                "fc_w": fc_host,
                "fc_be": be,
            }
        )
    return in_maps


def kernel(contexts, emb_W, emb_b, fc_W, fc_b):
    if "nc" not in _CACHE:
        _CACHE["nc"] = _build_nc()
    nc = _CACHE["nc"]
    in_maps = _prep_inputs(contexts, emb_W, emb_b, fc_W, fc_b)
    trace = bool(int(os.environ.get("KERNEL_TRACE", "0")))
    if trace:
        trace = _install_trace_hook()
    tc_env = os.environ.get("KERNEL_TRACE_CORES")
    kw = {}
    if tc_env:
        kw["trace_cores"] = [int(x) for x in tc_env.split(",")]
    res = run_bass_kernel_spmd(
        nc, in_maps, core_ids=list(range(NCORES)), trace=trace, **kw
    )
    _CACHE["mean_exec_time_ns"] = res.mean_exec_time_ns
    _CACHE["max_core"] = res.max_exec_time_core_id
    _CACHE["last_exec_time_ns"] = res.exec_time_ns
    dev = np.concatenate(
        [
            np.asarray(r["out"]).astype(np.float32).reshape(VSHARD, BATCH)
            for r in res.results
        ],
        axis=0,
    )[:VOCAB]
    return np.ascontiguousarray(dev.T)


# revision 40
# speedup vs baseline: 1.0161x; 1.0161x over previous
"""CBOW forward kernel for one TRN2 chip (8 NeuronCores), tensor-parallel on vocab.

Math (matches the reference):
    embed[b, c, :] = emb_W.T[contexts[b, c]] + emb_b          # gather
    out = embed.reshape(B, CTX*EMB) @ fc_W.T + fc_b           # [B, VOCAB]

No collectives: the vocab dim is sharded 8 ways (fc_W rows / fc_b /
output rows); the emb table is replicated and EVERY core gathers the FULL
batch locally, so there is no AllGather (the CC engine's ~47us mesh-init
made a collective-based prologue idle until ~90us).

Per-core device schedule:
  1. fc_W (fp8, 6.4 MiB) DMAs into SBUF in 7 chunks and stays resident.
     Chunk 0 loads up front; 1-6 are staged mid-kernel (from the gpsimd
     gather stream and after the first output DMAs) because early bulk
     loads clog the DMA queues and delay gather-data arrival by ~5-8us.
  2. The batch (2048 cols) is processed in 6 chunks of 2/2/3/3/3/3
     128-col blocks.  Per chunk: one indirect-DMA gather per (block, ctx)
     -- the hw consumes ONE index per partition per call, ~1.4us each on
     gpsimd, which paces the whole kernel -- pulls bf16 emb rows
     (pre-scaled by 512 on host); PE transposes (bf16, 1 cyc/row) + DVE
     casts produce the K-major fp8 rhs; then 98 vocab tiles x 2 fp8
     DoubleRow matmuls (K=512 as 2x256) accumulate in PSUM.  Chunk c+1's
     gathers overlap chunk c's matmuls; short chunks first (early matmul
     start) and short chunks last (less matmul work left when the gather
     stream ends at ~191us) minimize the makespan.  A burst of garbage
     warmup matmuls keeps the PE clock ramping before chunk 0 lands.
  3. PSUM drain (alternating scalar/vector) fuses the 2^-18 descale and
     bias add into bf16; output DMAs are batched 7 vocab tiles at a time
     (sync queue) into a [98, 128, 2048] DRAM layout that reshapes to
     [VSHARD, BATCH] on the host with no transpose; the final groups are
     split so the last DMAs start draining before the last drains finish.

Both matmul operands are pre-scaled by 512 (2^9) so fp8e4 stays out of
its denormal range; emb_b and fc_b are folded on the host into one
effective f32 bias fc_be = fc_W @ tile(emb_b, CTX) + fc_b.
"""

import os

import numpy as np

import concourse.bacc as bacc
import concourse.bass as bass
import concourse.mybir as mybir
import concourse.tile as tile
from concourse.bass_utils import run_bass_kernel_spmd

# Problem shape (hardcoded per harness contract).
VOCAB = 100000
CTX = 8
EMB = 64
BATCH = 2048
K = CTX * EMB            # 512 contraction dim
NCORES = 8
VSHARD = 12544           # 98 * 128, vocab rows per core (padded)
VPAD = VSHARD * NCORES   # 100352
NVT = VSHARD // 128      # 98 vocab tiles per core
NCHUNK = 7               # fc_W DMA chunks
VT_PER_CHUNK = NVT // NCHUNK   # 14 vocab tiles per chunk
CHUNK_COLS = VT_PER_CHUNK * 128  # 1792

# batch chunks: 128-col blocks each.  The gather is 1 indirect call per
# (block, ctx) at ~1.4us on gpsimd (the hw reads ONE index per partition
# per call), so the stream paces the pipeline; two short chunks up front
# start the matmuls early, 512-col chunks later maximize N-efficiency.
BCHUNKS = [
    (0, 1),
    (2, 3),
    (4, 5, 6),
    (7, 8, 9),
    (10, 11, 12),
    (13, 14, 15),
]
NWARM = 24               # idle-PE warmup matmuls while chunk 0 gathers land

SCALE = 512.0            # 2^9 per fp8 operand (avoid e4m3 denormals)
DESCALE = 2.0 ** -18     # undone in the PSUM drain

F32 = mybir.dt.float32
BF16 = mybir.dt.bfloat16
I32 = mybir.dt.int32
FP8 = mybir.dt.float8e4
NP_FP8 = mybir.dt.np(FP8)
NP_BF16 = mybir.dt.np(BF16)
OUT_DT = BF16            # output quantization: rel err ~1.4e-2 < 2e-2 gate

_CACHE = {}


def _install_trace_hook():
    """Provide the missing antenv.axon_hooks module so trace=True works.

    The agent image's antenv lacks axon_hooks; recreate it and install the
    ctypes NTFF hook from trn_boot.  Degrades silently on any failure.
    """
    import sys
    import types

    try:
        if "antenv.axon_hooks" not in sys.modules:
            mod = types.ModuleType("antenv.axon_hooks")
            mod._hook = None
            mod.set_axon_ntff_profile_hook = lambda h: setattr(mod, "_hook", h)
            mod.get_axon_ntff_profile_hook = lambda: mod._hook
            sys.modules["antenv.axon_hooks"] = mod
            import antenv

            antenv.axon_hooks = mod
        mod = sys.modules["antenv.axon_hooks"]
        if mod.get_axon_ntff_profile_hook() is None:
            if "/root/.axon_site/trn_agent_boot" not in sys.path:
                sys.path.insert(0, "/root/.axon_site/trn_agent_boot")
            import trn_boot

            mod.set_axon_ntff_profile_hook(
                trn_boot._ntff_profile_via_ctypes("/opt/axon/libaxon_pjrt.so")
            )
        return True
    except Exception as e:  # pragma: no cover
        print(f"trace hook install failed: {type(e).__name__}: {e}")
        return False


def _build_nc():
    nc = bacc.Bacc(
        "TRN2", target_bir_lowering=False, debug=False, num_devices=NCORES
    )
    # idx_my[p, j] = contexts[(j//8)*128 + p, j%8]; col block (j//8), ctx j%8
    idx_my = nc.declare_dram_parameter("idx_my", [128, 128], I32, isOutput=False)
    emb_wt = nc.declare_dram_parameter("emb_wt", [VOCAB, EMB], BF16, isOutput=False)
    ident_d = nc.declare_dram_parameter("ident_d", [128, 128], BF16, isOutput=False)
    fc_w = nc.declare_dram_parameter(
        "fc_w", [NCHUNK, 128, 4, CHUNK_COLS], FP8, isOutput=False
    )
    fc_be = nc.declare_dram_parameter("fc_be", [128, NVT], F32, isOutput=False)
    # out[v, p, d] = logits_scaled[vocab row v*128+p, batch col d]
    out = nc.declare_dram_parameter("out", [NVT, 128, BATCH], OUT_DT, isOutput=True)

    DR = mybir.MatmulPerfMode.DoubleRow

    with tile.TileContext(nc) as tc:
        with tc.tile_pool(name="const", bufs=1) as const:
            idx_sb = const.tile([128, 128], I32, tag="idx", name="idx_sb")
            nc.sync.dma_start(out=idx_sb[:], in_=idx_my[:])
            ident = const.tile([128, 128], BF16, tag="ident", name="ident")
            nc.sync.dma_start(out=ident[:], in_=ident_d[:])
            fcbe_sb = const.tile([128, NVT], F32, tag="fcbe", name="fcbe_sb")
            nc.sync.dma_start(out=fcbe_sb[:], in_=fc_be[:])
            # fc_W fp8 shard: fully SBUF-resident (7 x 917 KiB chunks).
            # fcw[ci][p, ksub, col] = fc_W.T_scaled[ksub*128+p, ci*1792+col]
            # Chunks 0-2 load up front; 3-6 when the first matmul chunk
            # starts, so the gather descriptors aren't stuck behind 6.4 MiB.
            fcw = []
            for ci in range(NCHUNK):
                t = const.tile(
                    [128, 4, CHUNK_COLS], FP8, tag=f"fcw{ci}", name=f"fcw{ci}"
                )
                if ci < 1:
                    nc.scalar.dma_start(out=t[:], in_=fc_w[ci])
                fcw.append(t)

            # warm the ACT Identity table before the main loop needs it
            actwarm = const.tile([128, 1], F32, tag="actwarm", name="actwarm")
            nc.scalar.activation(
                out=actwarm[:],
                in_=fcbe_sb[:, 0:1],
                func=mybir.ActivationFunctionType.Identity,
                bias=fcbe_sb[:, 0:1],
            )

            with (
                tc.tile_pool(name="rawp", bufs=3) as rawp,
                tc.tile_pool(name="embp", bufs=3) as embp,
                tc.tile_pool(name="tpsum", bufs=2, space="PSUM") as tpp,
                tc.tile_pool(name="mpsum", bufs=6, space="PSUM") as mps,
                tc.tile_pool(name="outp", bufs=4) as osbp,
            ):
                # PE p-state warmup: one PSUM accumulation group of garbage
                # bf16 matmuls keeps the tensor clock ramping while chunk 0's
                # gather data lands.  Never read back.
                wps = mps.tile([128, 512], F32, tag="mps", name="mps")
                for i in range(NWARM):
                    nc.tensor.matmul(
                        out=wps[:, 0:128],
                        lhsT=ident[:],
                        rhs=ident[:],
                        start=(i == 0),
                        stop=(i == NWARM - 1),
                    )
                col0 = 0
                for c, mms in enumerate(BCHUNKS):
                    nmm = len(mms)
                    ncols = 128 * nmm
                    # gather: raw[p, (gi*8+cc)*64 + e] = emb_scaled[contexts[
                    #   (mms[gi])*128 + p, cc], e].  One row per partition
                    #   per call: the hw reads a single index per partition
                    #   and copies a contiguous run, so multi-index offset
                    #   APs silently gather rows base..base+k-1 instead.
                    raw = rawp.tile([128, 32 * EMB], BF16, tag="raw", name="raw")
                    for gi, mm in enumerate(mms):
                        for cc in range(8):
                            j = mm * 8 + cc
                            nc.gpsimd.indirect_dma_start(
                                out=raw[:, (gi * 8 + cc) * EMB : (gi * 8 + cc + 1) * EMB],
                                out_offset=None,
                                in_=emb_wt[:],
                                in_offset=bass.IndirectOffsetOnAxis(
                                    ap=idx_sb[:, j : j + 1], axis=0
                                ),
                            )
                            if c == 0 and gi * 8 + cc in (11, 15):
                                # fcw 1-2 issue mid-gather-stream from this
                                # queue: early enough for the matmul deadline,
                                # late enough not to clog the DMA queues that
                                # chunk 0/1's gather data drains through.
                                cl = 1 if gi * 8 + cc == 11 else 2
                                nc.gpsimd.dma_start(out=fcw[cl][:], in_=fc_w[cl])
                    # transpose to K-major fp8:
                    # embT[j][q, i, gi*128+p] = emb_scaled.T[(2j+i)*128+q,
                    #   batch col mms[gi]*128+p]
                    embT = [
                        embp.tile(
                            [128, 2, 512], FP8, tag=f"eT{j}", name=f"embT{j}"
                        )
                        for j in range(2)
                    ]
                    for gi in range(nmm):
                        for t in range(4):
                            psT = tpp.tile(
                                [128, 1024], BF16, tag="psT", name="psT"
                            )
                            nc.tensor.transpose(
                                psT[:, 0:128],
                                raw[
                                    :,
                                    (gi * 8 + 2 * t) * EMB
                                    : (gi * 8 + 2 * t + 2) * EMB,
                                ],
                                ident[:],
                            )
                            nc.vector.tensor_copy(
                                out=embT[t // 2][
                                    :, t % 2, gi * 128 : (gi + 1) * 128
                                ],
                                in_=psT[:, 0:128],
                            )


                    # matmul sweep: 98 vocab tiles x 2 fp8 DR matmuls
                    last_chunk = c == len(BCHUNKS) - 1
                    for vg in range(NVT // 7):
                        osb = osbp.tile(
                            [128, 7, 512], OUT_DT, tag="osb", name="osb"
                        )
                        split = last_chunk and vg >= 12
                        for vv in range(7):
                            v = vg * 7 + vv
                            ci, vt = divmod(v, VT_PER_CHUNK)
                            ps = mps.tile([128, 512], F32, tag="mps", name="mps")
                            for j in range(2):
                                nc.tensor.matmul(
                                    out=ps[:, 0:ncols],
                                    lhsT=fcw[ci][
                                        :, 2 * j : 2 * j + 2,
                                        vt * 128 : (vt + 1) * 128,
                                    ],
                                    rhs=embT[j][:, :, 0:ncols],
                                    start=(j == 0),
                                    stop=(j == 1),
                                    perf_mode=DR,
                                )
                            if (v + c) % 2 == 0:
                                nc.scalar.activation(
                                    out=osb[:, vv, 0:ncols],
                                    in_=ps[:, 0:ncols],
                                    func=mybir.ActivationFunctionType.Identity,
                                    bias=fcbe_sb[:, v : v + 1],
                                    scale=DESCALE,
                                )

                            else:
                                nc.vector.tensor_scalar(
                                    out=osb[:, vv, 0:ncols],
                                    in0=ps[:, 0:ncols],
                                    scalar1=DESCALE,
                                    scalar2=fcbe_sb[:, v : v + 1],
                                    op0=mybir.AluOpType.mult,
                                    op1=mybir.AluOpType.add,
                                )
                            if split and vv == 3:
                                # kernel-tail trim: fire the first half of the
                                # final output groups as soon as 4 drains land
                                nc.sync.dma_start(
                                    out=out[
                                        vg * 7 : vg * 7 + 4, :,
                                        col0 : col0 + ncols,
                                    ].rearrange("v p d -> p v d"),
                                    in_=osb[:, 0:4, 0:ncols],
                                )
                        nc.sync.dma_start(
                            out=out[
                                vg * 7 + (4 if split else 0) : vg * 7 + 7,
                                :, col0 : col0 + ncols,
                            ].rearrange("v p d -> p v d"),
                            in_=osb[:, (4 if split else 0) : 7, 0:ncols],
                        )
                        if c == 0 and vg < 4:
                            # fcw 3-6 stream after chunk 0's first output
                            # DMAs -- paced issue keeps the queues clear for
                            # the ongoing gather stream.
                            nc.sync.dma_start(
                                out=fcw[3 + vg][:], in_=fc_w[3 + vg]
                            )
                    col0 += ncols
    nc.compile()
    return nc


def _prep_inputs(contexts, emb_W, emb_b, fc_W, fc_b):
    contexts = np.asarray(contexts)
    emb_W = np.asarray(emb_W, dtype=np.float32)
    emb_b = np.asarray(emb_b, dtype=np.float32)
    fc_W = np.asarray(fc_W, dtype=np.float32)
    fc_b = np.asarray(fc_b, dtype=np.float32)

    # idx2d[j, p] = contexts[(j//8)*128 + p, j%8]; device gathers column
    # block j//8 (128 batch rows), ctx position j%8, natural batch order
    idx2d = (
        contexts.astype(np.int64).reshape(BATCH // 128, 128, CTX)
        .transpose(0, 2, 1).reshape(BATCH // 128 * CTX, 128)
    )
    idx_all = np.ascontiguousarray(idx2d.T.astype(np.int32))  # [128, 128]
    emb_wt = np.ascontiguousarray((emb_W.T * SCALE).astype(NP_BF16))
    ident = np.eye(128, dtype=np.float32).astype(NP_BF16)

    # effective bias: fc_be = fc_W @ tile(emb_b, CTX) + fc_b  (padded, true scale)
    emb_b_t = np.tile(emb_b, CTX)
    fc_be_full = (
        fc_W.astype(np.float64) @ emb_b_t.astype(np.float64)
        + fc_b.astype(np.float64)
    ).astype(np.float32)
    fc_be_pad = np.zeros(VPAD, dtype=np.float32)
    fc_be_pad[:VOCAB] = fc_be_full

    # fc_W.T padded to VPAD cols, scaled into fp8 range, chunked per-core
    fcT = np.zeros((K, VPAD), dtype=np.float32)
    fcT[:, :VOCAB] = fc_W.T
    fcT = np.clip(fcT * SCALE, -240.0, 240.0).astype(NP_FP8)

    in_maps = []
    for s in range(NCORES):
        shard = fcT[:, s * VSHARD : (s + 1) * VSHARD]
        fc_host = np.ascontiguousarray(
            shard.reshape(4, 128, NCHUNK, CHUNK_COLS).transpose(2, 1, 0, 3)
        )
        be = np.ascontiguousarray(
            fc_be_pad[s * VSHARD : (s + 1) * VSHARD].reshape(NVT, 128).T
        )
        in_maps.append(
            {
                "idx_my": idx_all,
                "emb_wt": emb_wt,
                "ident_d": ident,
                "fc_w": fc_host,
                "fc_be": be,
            }
        )
    return in_maps


def kernel(contexts, emb_W, emb_b, fc_W, fc_b):
    if "nc" not in _CACHE:
        _CACHE["nc"] = _build_nc()
    nc = _CACHE["nc"]
    in_maps = _prep_inputs(contexts, emb_W, emb_b, fc_W, fc_b)
    trace = bool(int(os.environ.get("KERNEL_TRACE", "0")))
    if trace:
        trace = _install_trace_hook()
    tc_env = os.environ.get("KERNEL_TRACE_CORES")
    kw = {}
    if tc_env:
        kw["trace_cores"] = [int(x) for x in tc_env.split(",")]
    res = run_bass_kernel_spmd(
        nc, in_maps, core_ids=list(range(NCORES)), trace=trace, **kw
    )
    _CACHE["mean_exec_time_ns"] = res.mean_exec_time_ns
    _CACHE["max_core"] = res.max_exec_time_core_id
    _CACHE["last_exec_time_ns"] = res.exec_time_ns
    dev = np.concatenate(
        [
            np.asarray(r["out"]).astype(np.float32).reshape(VSHARD, BATCH)
            for r in res.results
        ],
        axis=0,
    )[:VOCAB]
    return np.ascontiguousarray(dev.T)


# revision 42
# speedup vs baseline: 1.0371x; 1.0207x over previous
"""CBOW forward kernel for one TRN2 chip (8 NeuronCores), tensor-parallel on vocab.

Math (matches the reference):
    embed[b, c, :] = emb_W.T[contexts[b, c]] + emb_b          # gather
    out = embed.reshape(B, CTX*EMB) @ fc_W.T + fc_b           # [B, VOCAB]

No collectives: the vocab dim is sharded 8 ways (fc_W rows / fc_b /
output rows); the emb table is replicated and EVERY core gathers the FULL
batch locally, so there is no AllGather (the CC engine's ~47us mesh-init
made a collective-based prologue idle until ~90us).

Per-core device schedule:
  1. fc_W (fp8, 6.4 MiB) DMAs into SBUF in 7 chunks and stays resident.
     Chunk 0 loads up front; 1-6 are staged mid-kernel (from the gpsimd
     gather stream and after the first output DMAs) because early bulk
     loads clog the DMA queues and delay gather-data arrival by ~5-8us.
  2. The batch (2048 cols) is processed in 6 chunks of 2/2/3/3/3/3
     128-col blocks.  Per chunk: one indirect-DMA gather per (block, ctx)
     -- the hw consumes ONE index per partition per call, ~1.4us each on
     gpsimd, which paces the whole kernel -- pulls bf16 emb rows
     (pre-scaled by 512 on host); PE transposes (bf16, 1 cyc/row) + DVE
     casts produce the K-major fp8 rhs; then 98 vocab tiles x 2 fp8
     DoubleRow matmuls (K=512 as 2x256) accumulate in PSUM.  Chunk c+1's
     gathers overlap chunk c's matmuls; short chunks first (early matmul
     start) and short chunks last (less matmul work left when the gather
     stream ends at ~191us) minimize the makespan.  A burst of garbage
     warmup matmuls keeps the PE clock ramping before chunk 0 lands.
  3. PSUM drain (alternating scalar/vector) fuses the 2^-18 descale and
     bias add into bf16; output DMAs are batched 7 vocab tiles at a time
     (sync queue) into a [98, 128, 2048] DRAM layout that reshapes to
     [VSHARD, BATCH] on the host with no transpose; the final groups are
     split so the last DMAs start draining before the last drains finish.

Both matmul operands are pre-scaled by 512 (2^9) so fp8e4 stays out of
its denormal range; emb_b and fc_b are folded on the host into one
effective f32 bias fc_be = fc_W @ tile(emb_b, CTX) + fc_b.
"""

import os

import numpy as np

import concourse.bacc as bacc
import concourse.bass as bass
import concourse.mybir as mybir
import concourse.tile as tile
from concourse.bass_utils import run_bass_kernel_spmd

# Problem shape (hardcoded per harness contract).
VOCAB = 100000
CTX = 8
EMB = 64
BATCH = 2048
K = CTX * EMB            # 512 contraction dim
NCORES = 8
VSHARD = 12544           # 98 * 128, vocab rows per core (padded)
VPAD = VSHARD * NCORES   # 100352
NVT = VSHARD // 128      # 98 vocab tiles per core
NCHUNK = 7               # fc_W DMA chunks
VT_PER_CHUNK = NVT // NCHUNK   # 14 vocab tiles per chunk
CHUNK_COLS = VT_PER_CHUNK * 128  # 1792

# batch chunks: 128-col blocks each.  The gather is 1 indirect call per
# (block, ctx) at ~1.4us on gpsimd (the hw reads ONE index per partition
# per call), so the stream paces the pipeline; two short chunks up front
# start the matmuls early, 512-col chunks later maximize N-efficiency.
BCHUNKS = [
    (0, 1),
    (2, 3),
    (4, 5, 6),
    (7, 8, 9),
    (10, 11, 12),
    (13, 14, 15),
]
NWARM = 24               # idle-PE warmup matmuls while chunk 0 gathers land

SCALE = 512.0            # 2^9 per fp8 operand (avoid e4m3 denormals)
DESCALE = 2.0 ** -18     # undone in the PSUM drain

F32 = mybir.dt.float32
BF16 = mybir.dt.bfloat16
I32 = mybir.dt.int32
FP8 = mybir.dt.float8e4
NP_FP8 = mybir.dt.np(FP8)
NP_BF16 = mybir.dt.np(BF16)
OUT_DT = BF16            # output quantization: rel err ~1.4e-2 < 2e-2 gate

_CACHE = {}


def _install_trace_hook():
    """Provide the missing antenv.axon_hooks module so trace=True works.

    The agent image's antenv lacks axon_hooks; recreate it and install the
    ctypes NTFF hook from trn_boot.  Degrades silently on any failure.
    """
    import sys
    import types

    try:
        if "antenv.axon_hooks" not in sys.modules:
            mod = types.ModuleType("antenv.axon_hooks")
            mod._hook = None
            mod.set_axon_ntff_profile_hook = lambda h: setattr(mod, "_hook", h)
            mod.get_axon_ntff_profile_hook = lambda: mod._hook
            sys.modules["antenv.axon_hooks"] = mod
            import antenv

            antenv.axon_hooks = mod
        mod = sys.modules["antenv.axon_hooks"]
        if mod.get_axon_ntff_profile_hook() is None:
            if "/root/.axon_site/trn_agent_boot" not in sys.path:
                sys.path.insert(0, "/root/.axon_site/trn_agent_boot")
            import trn_boot

            mod.set_axon_ntff_profile_hook(
                trn_boot._ntff_profile_via_ctypes("/opt/axon/libaxon_pjrt.so")
            )
        return True
    except Exception as e:  # pragma: no cover
        print(f"trace hook install failed: {type(e).__name__}: {e}")
        return False


def _build_nc():
    nc = bacc.Bacc(
        "TRN2", target_bir_lowering=False, debug=False, num_devices=NCORES
    )
    # idx_my[p, j] = contexts[(j//8)*128 + p, j%8]; col block (j//8), ctx j%8
    idx_my = nc.declare_dram_parameter("idx_my", [128, 128], I32, isOutput=False)
    emb_wt = nc.declare_dram_parameter("emb_wt", [VOCAB, EMB], BF16, isOutput=False)
    ident_d = nc.declare_dram_parameter("ident_d", [128, 128], BF16, isOutput=False)
    fc_w = nc.declare_dram_parameter(
        "fc_w", [NCHUNK, 128, 4, CHUNK_COLS], FP8, isOutput=False
    )
    fc_be = nc.declare_dram_parameter("fc_be", [128, NVT], F32, isOutput=False)
    # out[v, p, d] = logits_scaled[vocab row v*128+p, batch col d]
    out = nc.declare_dram_parameter("out", [NVT, 128, BATCH], OUT_DT, isOutput=True)

    DR = mybir.MatmulPerfMode.DoubleRow

    with tile.TileContext(nc) as tc:
        with tc.tile_pool(name="const", bufs=1) as const:
            idx_sb = const.tile([128, 128], I32, tag="idx", name="idx_sb")
            nc.sync.dma_start(out=idx_sb[:], in_=idx_my[:])
            ident = const.tile([128, 128], BF16, tag="ident", name="ident")
            nc.sync.dma_start(out=ident[:], in_=ident_d[:])
            fcbe_sb = const.tile([128, NVT], F32, tag="fcbe", name="fcbe_sb")
            nc.sync.dma_start(out=fcbe_sb[:], in_=fc_be[:])
            # fc_W fp8 shard: fully SBUF-resident (7 x 917 KiB chunks).
            # fcw[ci][p, ksub, col] = fc_W.T_scaled[ksub*128+p, ci*1792+col]
            # Chunks 0-2 load up front; 3-6 when the first matmul chunk
            # starts, so the gather descriptors aren't stuck behind 6.4 MiB.
            fcw = []
            for ci in range(NCHUNK):
                t = const.tile(
                    [128, 4, CHUNK_COLS], FP8, tag=f"fcw{ci}", name=f"fcw{ci}"
                )
                if ci < 1:
                    nc.scalar.dma_start(out=t[:], in_=fc_w[ci])
                fcw.append(t)

            # warm the ACT Identity table before the main loop needs it
            actwarm = const.tile([128, 1], F32, tag="actwarm", name="actwarm")
            nc.scalar.activation(
                out=actwarm[:],
                in_=fcbe_sb[:, 0:1],
                func=mybir.ActivationFunctionType.Identity,
                bias=fcbe_sb[:, 0:1],
            )

            with (
                tc.tile_pool(name="rawp", bufs=3) as rawp,
                tc.tile_pool(name="embp", bufs=3) as embp,
                tc.tile_pool(name="tpsum", bufs=2, space="PSUM") as tpp,
                tc.tile_pool(name="mpsum", bufs=6, space="PSUM") as mps,
                tc.tile_pool(name="outp", bufs=4) as osbp,
            ):
                # PE p-state warmup: one PSUM accumulation group of garbage
                # bf16 matmuls keeps the tensor clock ramping while chunk 0's
                # gather data lands.  Never read back.  The source tile is
                # memset (not DMAed) so the warmup isn't stuck behind the
                # early parameter loads draining the DMA queues.
                wsrc = const.tile([128, 128], BF16, tag="wsrc", name="wsrc")
                nc.vector.memset(wsrc[:], 1.0)
                wps = mps.tile([128, 512], F32, tag="mps", name="mps")
                for i in range(NWARM):
                    nc.tensor.matmul(
                        out=wps[:, 0:128],
                        lhsT=wsrc[:],
                        rhs=wsrc[:],
                        start=(i == 0),
                        stop=(i == NWARM - 1),
                    )
                col0 = 0
                for c, mms in enumerate(BCHUNKS):
                    nmm = len(mms)
                    ncols = 128 * nmm
                    # gather: raw[p, (gi*8+cc)*64 + e] = emb_scaled[contexts[
                    #   (mms[gi])*128 + p, cc], e].  One row per partition
                    #   per call: the hw reads a single index per partition
                    #   and copies a contiguous run, so multi-index offset
                    #   APs silently gather rows base..base+k-1 instead.
                    raw = rawp.tile([128, 32 * EMB], BF16, tag="raw", name="raw")
                    for gi, mm in enumerate(mms):
                        for cc in range(8):
                            j = mm * 8 + cc
                            nc.gpsimd.indirect_dma_start(
                                out=raw[:, (gi * 8 + cc) * EMB : (gi * 8 + cc + 1) * EMB],
                                out_offset=None,
                                in_=emb_wt[:],
                                in_offset=bass.IndirectOffsetOnAxis(
                                    ap=idx_sb[:, j : j + 1], axis=0
                                ),
                            )
                            if c == 0 and gi * 8 + cc in (11, 15):
                                # fcw 1-2 issue mid-gather-stream from this
                                # queue: early enough for the matmul deadline,
                                # late enough not to clog the DMA queues that
                                # chunk 0/1's gather data drains through.
                                cl = 1 if gi * 8 + cc == 11 else 2
                                nc.gpsimd.dma_start(out=fcw[cl][:], in_=fc_w[cl])
                    # transpose to K-major fp8:
                    # embT[j][q, i, gi*128+p] = emb_scaled.T[(2j+i)*128+q,
                    #   batch col mms[gi]*128+p]
                    embT = [
                        embp.tile(
                            [128, 2, 512], FP8, tag=f"eT{j}", name=f"embT{j}"
                        )
                        for j in range(2)
                    ]
                    for gi in range(nmm):
                        for t in range(4):
                            psT = tpp.tile(
                                [128, 1024], BF16, tag="psT", name="psT"
                            )
                            nc.tensor.transpose(
                                psT[:, 0:128],
                                raw[
                                    :,
                                    (gi * 8 + 2 * t) * EMB
                                    : (gi * 8 + 2 * t + 2) * EMB,
                                ],
                                ident[:],
                            )
                            nc.vector.tensor_copy(
                                out=embT[t // 2][
                                    :, t % 2, gi * 128 : (gi + 1) * 128
                                ],
                                in_=psT[:, 0:128],
                            )


                    # matmul sweep: 98 vocab tiles x 2 fp8 DR matmuls
                    last_chunk = c == len(BCHUNKS) - 1
                    for vg in range(NVT // 7):
                        osb = osbp.tile(
                            [128, 7, 512], OUT_DT, tag="osb", name="osb"
                        )
                        split = last_chunk and vg >= 9
                        for vv in range(7):
                            v = vg * 7 + vv
                            ci, vt = divmod(v, VT_PER_CHUNK)
                            ps = mps.tile([128, 512], F32, tag="mps", name="mps")
                            for j in range(2):
                                nc.tensor.matmul(
                                    out=ps[:, 0:ncols],
                                    lhsT=fcw[ci][
                                        :, 2 * j : 2 * j + 2,
                                        vt * 128 : (vt + 1) * 128,
                                    ],
                                    rhs=embT[j][:, :, 0:ncols],
                                    start=(j == 0),
                                    stop=(j == 1),
                                    perf_mode=DR,
                                )
                            if (v + c) % 2 == 0:
                                nc.scalar.activation(
                                    out=osb[:, vv, 0:ncols],
                                    in_=ps[:, 0:ncols],
                                    func=mybir.ActivationFunctionType.Identity,
                                    bias=fcbe_sb[:, v : v + 1],
                                    scale=DESCALE,
                                )

                            else:
                                nc.vector.tensor_scalar(
                                    out=osb[:, vv, 0:ncols],
                                    in0=ps[:, 0:ncols],
                                    scalar1=DESCALE,
                                    scalar2=fcbe_sb[:, v : v + 1],
                                    op0=mybir.AluOpType.mult,
                                    op1=mybir.AluOpType.add,
                                )
                            if split and vv == 3:
                                # kernel-tail trim: fire the first half of the
                                # final output groups as soon as 4 drains land
                                nc.sync.dma_start(
                                    out=out[
                                        vg * 7 : vg * 7 + 4, :,
                                        col0 : col0 + ncols,
                                    ].rearrange("v p d -> p v d"),
                                    in_=osb[:, 0:4, 0:ncols],
                                )
                        nc.sync.dma_start(
                            out=out[
                                vg * 7 + (4 if split else 0) : vg * 7 + 7,
                                :, col0 : col0 + ncols,
                            ].rearrange("v p d -> p v d"),
                            in_=osb[:, (4 if split else 0) : 7, 0:ncols],
                        )
                        if c == 0 and vg < 4:
                            # fcw 3-6 stream after chunk 0's first output
                            # DMAs -- paced issue keeps the queues clear for
                            # the ongoing gather stream.
                            nc.sync.dma_start(
                                out=fcw[3 + vg][:], in_=fc_w[3 + vg]
                            )
                    col0 += ncols
    nc.compile()
    return nc


def _prep_inputs(contexts, emb_W, emb_b, fc_W, fc_b):
    contexts = np.asarray(contexts)
    emb_W = np.asarray(emb_W, dtype=np.float32)
    emb_b = np.asarray(emb_b, dtype=np.float32)
    fc_W = np.asarray(fc_W, dtype=np.float32)
    fc_b = np.asarray(fc_b, dtype=np.float32)

    # idx2d[j, p] = contexts[(j//8)*128 + p, j%8]; device gathers column
    # block j//8 (128 batch rows), ctx position j%8, natural batch order
    idx2d = (
        contexts.astype(np.int64).reshape(BATCH // 128, 128, CTX)
        .transpose(0, 2, 1).reshape(BATCH // 128 * CTX, 128)
    )
    idx_all = np.ascontiguousarray(idx2d.T.astype(np.int32))  # [128, 128]
    emb_wt = np.ascontiguousarray((emb_W.T * SCALE).astype(NP_BF16))
    ident = np.eye(128, dtype=np.float32).astype(NP_BF16)

    # effective bias: fc_be = fc_W @ tile(emb_b, CTX) + fc_b  (padded, true scale)
    emb_b_t = np.tile(emb_b, CTX)
    fc_be_full = (
        fc_W.astype(np.float64) @ emb_b_t.astype(np.float64)
        + fc_b.astype(np.float64)
    ).astype(np.float32)
    fc_be_pad = np.zeros(VPAD, dtype=np.float32)
    fc_be_pad[:VOCAB] = fc_be_full

    # fc_W.T padded to VPAD cols, scaled into fp8 range, chunked per-core
    fcT = np.zeros((K, VPAD), dtype=np.float32)
    fcT[:, :VOCAB] = fc_W.T
    fcT = np.clip(fcT * SCALE, -240.0, 240.0).astype(NP_FP8)

    in_maps = []
    for s in range(NCORES):
        shard = fcT[:, s * VSHARD : (s + 1) * VSHARD]
        fc_host = np.ascontiguousarray(
            shard.reshape(4, 128, NCHUNK, CHUNK_COLS).transpose(2, 1, 0, 3)
        )
        be = np.ascontiguousarray(
            fc_be_pad[s * VSHARD : (s + 1) * VSHARD].reshape(NVT, 128).T
        )
        in_maps.append(
            {
                "idx_my": idx_all,
                "emb_wt": emb_wt,
                "ident_d": ident,
                "fc_w": fc_host,
                "fc_be": be,
            }
        )
    return in_maps


def kernel(contexts, emb_W, emb_b, fc_W, fc_b):
    if "nc" not in _CACHE:
        _CACHE["nc"] = _build_nc()
    nc = _CACHE["nc"]
    in_maps = _prep_inputs(contexts, emb_W, emb_b, fc_W, fc_b)
    trace = bool(int(os.environ.get("KERNEL_TRACE", "0")))
    if trace:
        trace = _install_trace_hook()
    tc_env = os.environ.get("KERNEL_TRACE_CORES")
    kw = {}
    if tc_env:
        kw["trace_cores"] = [int(x) for x in tc_env.split(",")]
    res = run_bass_kernel_spmd(
        nc, in_maps, core_ids=list(range(NCORES)), trace=trace, **kw
    )
    _CACHE["mean_exec_time_ns"] = res.mean_exec_time_ns
    _CACHE["max_core"] = res.max_exec_time_core_id
    _CACHE["last_exec_time_ns"] = res.exec_time_ns
    dev = np.concatenate(
        [
            np.asarray(r["out"]).astype(np.float32).reshape(VSHARD, BATCH)
            for r in res.results
        ],
        axis=0,
    )[:VOCAB]
    return np.ascontiguousarray(dev.T)
